# revision 1
# baseline (speedup 1.0000x reference)
"""Trainium2 Bass kernel for nn_AttentionMechanism (batched attention with
per-sample queries), data-parallel across 8 NeuronCores.

Math (per batch row b):
    q = msgs @ Wq.T + bq                         [H]
    k_t = Wk @ tau_t + bk ; scores_t = q.k_t/32
    alpha = softmax(scores) ; out = sum_t alpha_t (Wv @ tau_t + bv)

Rewrite used here (exact up to softmax shift invariance):
    qk   = (msgs @ Wq.T + bq) @ Wk  = msgs @ (Wq.T @ Wk) + bq @ Wk   [TAU]
    scores_t = qk . tau_t / 32      (the q.bk term is constant in t -> cancels)
    p_t  = exp(scores_t)            (scores are O(1), no max-subtraction needed)
    ctx  = sum_t p_t tau_t / sum_t p_t
    out  = ctx @ Wv.T + bv          (uses sum alpha = 1)

This removes the [B,T,H] k-projection (17 GFLOP/core) and [B,T,VDIM]
v-projection entirely; the kernel streams tau once from HBM (32 MB/core).
"""

import math

import numpy as np

import concourse.bass as bass
import concourse.bacc as bacc
import concourse.tile as tile
from concourse import mybir
from concourse.bass_utils import run_bass_kernel_spmd
from concourse.masks import make_identity

F32 = mybir.dt.float32
BF16 = mybir.dt.bfloat16

B = 2048
T = 32
TAU = 1024
MSG = 512
HID = 1024
VDIM = 128
N_CORES = 8
B_LOCAL = B // N_CORES

Alu = mybir.AluOpType
Act = mybir.ActivationFunctionType


def build(b_local=B_LOCAL, t_chunk=8, chunk_bufs=4, prod_bufs=2,
          ts_act_per_chunk=0, sum_engine="scalar", dma_cast=True,
          bcast_mult=True):
    assert b_local % 128 == 0 and T % t_chunk == 0
    n_btiles = b_local // 128
    n_chunks = T // t_chunk
    chunk_free = t_chunk * TAU

    nc = bacc.Bacc("TRN2", target_bir_lowering=False, debug=False)

    traj = nc.declare_dram_parameter(
        "imagined_trajectory", [b_local, T * TAU], F32, isOutput=False
    )
    msgs = nc.declare_dram_parameter(
        "received_messages", [b_local, MSG], F32, isOutput=False
    )
    Wq = nc.declare_dram_parameter("Wq", [HID, MSG], F32, isOutput=False)
    bq = nc.declare_dram_parameter("bq", [HID], F32, isOutput=False)
    Wk = nc.declare_dram_parameter("Wk", [HID, TAU], F32, isOutput=False)
    Wv = nc.declare_dram_parameter("Wv", [VDIM, TAU], F32, isOutput=False)
    bv = nc.declare_dram_parameter("bv", [VDIM], F32, isOutput=False)
    out = nc.declare_dram_parameter("out", [b_local, VDIM], F32, isOutput=True)

    HQ = HID // 128  # 8 h-chunks
    MQ = MSG // 128  # 4 m-chunks
    CQ = TAU // 128  # 8 c-chunks

    with tile.TileContext(nc) as tc:
        with (
            tc.tile_pool(name="const", bufs=1) as const,
            tc.tile_pool(name="persist", bufs=1) as persist,
            tc.tile_pool(name="psum", bufs=2, space="PSUM") as psum,
            tc.tile_pool(name="psum_ctx", bufs=1, space="PSUM") as psum_ctx,
            tc.tile_pool(name="psum_tr", bufs=2, space="PSUM") as psum_tr,
        ):
            ident_f = const.tile([128, 128], F32)
            make_identity(nc, ident_f)
            ident_b = const.tile([128, 128], BF16)
            make_identity(nc, ident_b)
            onespad_b = const.tile([128, 128], BF16)
            nc.vector.memset(onespad_b, 0.0)
            nc.vector.memset(onespad_b[0:1, :], 1.0)
            bv_sb = const.tile([1, VDIM], F32)
            nc.sync.dma_start(out=bv_sb, in_=bv[None, :])
            bvpad_b = const.tile([128, VDIM], BF16)
            nc.vector.memset(bvpad_b, 0.0)
            nc.vector.tensor_copy(out=bvpad_b[0:1, :], in_=bv_sb)
            WvT_b = persist.tile([128, CQ, VDIM], BF16)  # [c-part, c-chunk, d]
            qk_b = [
                persist.tile([128, TAU], BF16, tag=f"qkb{i}", name=f"qkb{i}")
                for i in range(n_btiles)
            ]

            # ---------- setup: weights, fused projection, per-sample qk ----------
            with tc.tile_pool(name="wtmp", bufs=1) as wtmp:
                Wq_b = wtmp.tile([128, HQ, MSG], BF16)  # [h-part, h-chunk, m]
                nc.gpsimd.dma_start(
                    out=Wq_b, in_=Wq[:, :].rearrange("(j p) m -> p j m", p=128)
                )
                Wk_b = wtmp.tile([128, HQ, TAU], BF16)  # [h-part, h-chunk, c]
                nc.gpsimd.dma_start(
                    out=Wk_b, in_=Wk[:, :].rearrange("(j p) c -> p j c", p=128)
                )

                # bq -> [h-part, h-chunk]
                bq_b = wtmp.tile([128, HQ], BF16)
                nc.gpsimd.dma_start(
                    out=bq_b, in_=bq[:].rearrange("(j p) -> p j", p=128)
                )

                # Wv [VDIM=128, TAU] -> WvT blocks [c-part, d]
                Wv_sb = wtmp.tile([VDIM, TAU], F32)
                nc.sync.dma_start(out=Wv_sb, in_=Wv[:, :])
                for j in range(CQ):
                    pt = psum_tr.tile([128, 128], F32, tag="tr", name="pt")
                    nc.tensor.transpose(pt, Wv_sb[:, j * 128 : (j + 1) * 128], ident_f)
                    nc.scalar.copy(out=WvT_b[:, j, :], in_=pt)

                # msgs -> msgsT blocks [m-part, b]
                msgsT_b = wtmp.tile([128, MQ, b_local], BF16)
                for bi in range(n_btiles):
                    ms = wtmp.tile([128, MSG], F32, tag="msgs_f32")
                    nc.sync.dma_start(
                        out=ms, in_=msgs[bi * 128 : (bi + 1) * 128, :]
                    )
                    for mi in range(MQ):
                        pt = psum_tr.tile([128, 128], F32, tag="tr", name="pt")
                        nc.tensor.transpose(
                            pt, ms[:, mi * 128 : (mi + 1) * 128], ident_f
                        )
                        nc.scalar.copy(
                            out=msgsT_b[:, mi, bi * 128 : (bi + 1) * 128], in_=pt
                        )

                # Wfused[m, c] = sum_h Wq[h, m] * Wk[h, c]   (= Wq.T @ Wk)
                Wfused_b = wtmp.tile([128, MQ, TAU], BF16)
                for mi in range(MQ):
                    pf = psum.tile([128, TAU], F32, tag="mm")
                    for nh in range(2):
                        nsl = slice(nh * 512, (nh + 1) * 512)
                        for j in range(HQ):
                            nc.tensor.matmul(
                                pf[:, nsl],
                                lhsT=Wq_b[:, j, mi * 128 : (mi + 1) * 128],
                                rhs=Wk_b[:, j, nsl],
                                start=(j == 0),
                                stop=(j == HQ - 1),
                            )
                    nc.scalar.copy(out=Wfused_b[:, mi, :], in_=pf)

                # qk_bias[c] = sum_h bq[h] * Wk[h, c]
                pb = psum.tile([1, TAU], F32, tag="mm", name="pb")
                for nh in range(2):
                    nsl = slice(nh * 512, (nh + 1) * 512)
                    for j in range(HQ):
                        nc.tensor.matmul(
                            pb[:, nsl],
                            lhsT=bq_b[:, j : j + 1],
                            rhs=Wk_b[:, j, nsl],
                            start=(j == 0),
                            stop=(j == HQ - 1),
                        )
                wf_bias_b = wtmp.tile([128, TAU], BF16)
                nc.vector.memset(wf_bias_b, 0.0)
                nc.scalar.copy(out=wf_bias_b[0:1, :], in_=pb)
                ones_row_b = wtmp.tile([128, b_local], BF16)
                nc.vector.memset(ones_row_b, 0.0)
                nc.vector.memset(ones_row_b[0:1, :], 1.0)

                # qk[b, c] = msgs @ Wfused + qk_bias, scaled by 1/sqrt(H)
                for bi in range(n_btiles):
                    pq = psum.tile([128, TAU], F32, tag="mm")
                    for nh in range(2):
                        nsl = slice(nh * 512, (nh + 1) * 512)
                        for mi in range(MQ):
                            nc.tensor.matmul(
                                pq[:, nsl],
                                lhsT=msgsT_b[:, mi, bi * 128 : (bi + 1) * 128],
                                rhs=Wfused_b[:, mi, nsl],
                                start=(mi == 0),
                                stop=False,
                            )
                        nc.tensor.matmul(
                            pq[:, nsl],
                            lhsT=ones_row_b[:, bi * 128 : (bi + 1) * 128],
                            rhs=wf_bias_b[:, nsl],
                            start=False,
                            stop=True,
                        )
                    nc.scalar.mul(out=qk_b[bi], in_=pq, mul=1.0 / math.sqrt(HID))

            # ---------- main loop: stream tau, scores -> exp -> weighted sum ----
            with (
                tc.tile_pool(name="stream", bufs=chunk_bufs) as stream,
                tc.tile_pool(name="bfp", bufs=prod_bufs) as bfp,
                tc.tile_pool(name="aux", bufs=2) as aux,
                tc.tile_pool(name="outp", bufs=2) as outp,
            ):
                dump = aux.tile([128, TAU], BF16, tag="dump", name="dump", bufs=1)
                for bi in range(n_btiles):
                    bsl = slice(bi * 128, (bi + 1) * 128)
                    ctx_ps = psum_ctx.tile([128, TAU], F32, tag="ctxps", name="ctx_ps")
                    scores = aux.tile([128, T], F32, tag="scores", name="scores")
                    p_t = aux.tile([128, T], F32, tag="p", name="p_t")

                    for ci in range(n_chunks):
                        chunk_bf = stream.tile([128, chunk_free], BF16, tag="chunk", name="chunk_bf")
                        c0 = ci * chunk_free
                        if dma_cast:
                            nc.gpsimd.dma_start(
                                out=chunk_bf, in_=traj[bsl, c0 : c0 + chunk_free]
                            )
                        else:
                            chunk = bfp.tile([128, chunk_free], F32, tag="cf32", name="chunk")
                            nc.sync.dma_start(
                                out=chunk, in_=traj[bsl, c0 : c0 + chunk_free]
                            )
                            nc.scalar.copy(out=chunk_bf, in_=chunk)
                        prod = bfp.tile([128, chunk_free], BF16, tag="prod", name="prod")
                        if bcast_mult:
                            qk_rep = bass.AP(
                                tensor=qk_b[bi].tensor,
                                offset=qk_b[bi].offset,
                                ap=[qk_b[bi].ap[0], [0, t_chunk], [1, TAU]],
                            )
                            nc.vector.tensor_tensor(
                                out=prod,
                                in0=chunk_bf,
                                in1=qk_rep,
                                op=Alu.mult,
                            )
                        else:
                            for tt in range(t_chunk):
                                seg = slice(tt * TAU, (tt + 1) * TAU)
                                nc.vector.tensor_tensor(
                                    out=prod[:, seg],
                                    in0=chunk_bf[:, seg],
                                    in1=qk_b[bi],
                                    op=Alu.mult,
                                )
                        for tt in range(t_chunk):
                            col = ci * t_chunk + tt
                            seg = slice(tt * TAU, (tt + 1) * TAU)
                            if sum_engine == "scalar":
                                nc.scalar.activation(
                                    out=dump,
                                    in_=prod[:, seg],
                                    func=Act.Copy,
                                    accum_out=scores[:, col : col + 1],
                                )
                            else:
                                nc.vector.tensor_reduce(
                                    out=scores[:, col : col + 1],
                                    in_=prod[:, seg],
                                    axis=mybir.AxisListType.X,
                                    op=Alu.add,
                                )
                        csl = slice(ci * t_chunk, (ci + 1) * t_chunk)
                        nc.scalar.activation(
                            out=p_t[:, csl], in_=scores[:, csl], func=Act.Exp
                        )
                        for tt in range(t_chunk):
                            col = ci * t_chunk + tt
                            seg = slice(tt * TAU, (tt + 1) * TAU)
                            if tt < ts_act_per_chunk:
                                nc.scalar.activation(
                                    out=prod[:, seg],
                                    in_=chunk_bf[:, seg],
                                    func=Act.Copy,
                                    scale=p_t[:, col : col + 1],
                                )
                            else:
                                nc.vector.tensor_scalar(
                                    out=prod[:, seg],
                                    in0=chunk_bf[:, seg],
                                    scalar1=p_t[:, col : col + 1],
                                    scalar2=None,
                                    op0=Alu.mult,
                                )
                            first = ci == 0 and tt == 0
                            last = ci == n_chunks - 1 and tt == t_chunk - 1
                            for nh in range(2):
                                nsl = slice(
                                    tt * TAU + nh * 512, tt * TAU + (nh + 1) * 512
                                )
                                nc.tensor.matmul(
                                    ctx_ps[:, nh * 512 : (nh + 1) * 512],
                                    lhsT=ident_b,
                                    rhs=prod[:, nsl],
                                    start=first,
                                    stop=last,
                                )

                    # normalize, project: out = (ctx / sum p) @ Wv.T + bv
                    s_sum = aux.tile([128, 1], F32, tag="ssum", name="s_sum")
                    nc.vector.tensor_reduce(
                        out=s_sum, in_=p_t, axis=mybir.AxisListType.X, op=Alu.add
                    )
                    rinv = aux.tile([128, 1], F32, tag="rinv", name="rinv")
                    nc.vector.reciprocal(out=rinv, in_=s_sum)
                    ctxn_f = aux.tile([128, TAU], F32, tag="ctxn", name="ctxn_f")
                    nc.scalar.activation(
                        out=ctxn_f,
                        in_=ctx_ps,
                        func=Act.Copy,
                        scale=rinv,
                    )
                    ctxT_b = aux.tile([128, CQ, 128], BF16, tag="ctxT", name="ctxT_b")
                    for j in range(CQ):
                        ptb = psum_tr.tile([128, 128], F32, tag="tr", name="ptb")
                        nc.tensor.transpose(
                            ptb, ctxn_f[:, j * 128 : (j + 1) * 128], ident_f
                        )
                        nc.scalar.copy(out=ctxT_b[:, j, :], in_=ptb)
                    pm = psum.tile([128, VDIM], F32, tag="mm", name="pm")
                    for j in range(CQ):
                        nc.tensor.matmul(
                            pm,
                            lhsT=ctxT_b[:, j, :],
                            rhs=WvT_b[:, j, :],
                            start=(j == 0),
                            stop=False,
                        )
                    nc.tensor.matmul(
                        pm,
                        lhsT=onespad_b,
                        rhs=bvpad_b,
                        start=False,
                        stop=True,
                    )
                    msg_out = outp.tile([128, VDIM], F32, tag="msg", name="msg_out")
                    nc.scalar.copy(out=msg_out, in_=pm)
                    nc.sync.dma_start(out=out[bsl, :], in_=msg_out)

    nc.compile()
    return nc


_NC_CACHE = {}


def _get_nc():
    key = "default"
    if key not in _NC_CACHE:
        _NC_CACHE[key] = build()
    return _NC_CACHE[key]


def make_in_maps(imagined_trajectory, received_messages, Wq, bq, Wk, Wv, bv):
    bl = B_LOCAL
    in_maps = []
    for i in range(N_CORES):
        sl = slice(i * bl, (i + 1) * bl)
        in_maps.append(
            {
                "imagined_trajectory": np.ascontiguousarray(
                    imagined_trajectory[sl], dtype=np.float32
                ),
                "received_messages": np.ascontiguousarray(
                    received_messages[sl], dtype=np.float32
                ),
                "Wq": np.asarray(Wq, dtype=np.float32),
                "bq": np.asarray(bq, dtype=np.float32),
                "Wk": np.asarray(Wk, dtype=np.float32),
                "Wv": np.asarray(Wv, dtype=np.float32),
                "bv": np.asarray(bv, dtype=np.float32),
            }
        )
    return in_maps


def kernel(
    imagined_trajectory,
    received_messages,
    Wq,
    bq,
    Wk,
    bk,
    Wv,
    bv,
):
    nc = _get_nc()
    in_maps = make_in_maps(
        imagined_trajectory, received_messages, Wq, bq, Wk, Wv, bv
    )
    res = run_bass_kernel_spmd(nc, in_maps, list(range(N_CORES)))
    return np.concatenate([res.results[i]["out"] for i in range(N_CORES)], axis=0)



# revision 5
# speedup vs baseline: 1.1231x; 1.1231x over previous
"""Trainium2 Bass kernel for nn_AttentionMechanism (batched attention with
per-sample queries), data-parallel across 8 NeuronCores.

Math (per batch row b):
    q = msgs @ Wq.T + bq                         [H]
    k_t = Wk @ tau_t + bk ; scores_t = q.k_t/32
    alpha = softmax(scores) ; out = sum_t alpha_t (Wv @ tau_t + bv)

Rewrite used (exact up to softmax shift invariance):
    qk   = msgs @ (Wq.T @ Wk) + bq @ Wk          [TAU]   (q.bk const in t -> cancels)
    scores_t = qk . tau_t / 32
    p_t  = exp(scores_t)            (scores are O(1), no max-subtraction needed)
    ctx  = sum_t p_t tau_t / sum_t p_t
    out  = ctx @ Wv.T + bv          (uses sum alpha = 1)

Host precomputes the batch-independent weight products (Wfused = Wq.T @ Wk,
qk_bias = bq @ Wk, WvT = Wv.T) and packs them bf16, so the device streams tau
once from HBM (32 MB/core) plus ~1.5 MB of weights.

Device schedule per 128-row b-tile, per t-chunk of 8 trajectory steps:
  DMA   : chunk [128, 8*1024] f32->bf16 cast on load (~11.1 us, the bound)
  Vector: 1x broadcast mult prod = chunk * qk_rep; 5x tensor_reduce -> scores;
          1x broadcast mult -> diag(p_t) blocks [128, 8, 128]
  Scalar: 3x activation-accum -> scores; 2x exp
  PE    : 16x matmul ctx[:, bank] += diag(p_t) @ chunk_t  (p_t scaling folded
          into the matmul weights, so no separate p_t*tau elementwise pass)
"""

import math

import numpy as np
import ml_dtypes

import concourse.bass as bass
import concourse.bacc as bacc
import concourse.tile as tile
from concourse import mybir
from concourse.bass_utils import run_bass_kernel_spmd
from concourse.masks import make_identity

F32 = mybir.dt.float32
BF16 = mybir.dt.bfloat16
NP_BF16 = ml_dtypes.bfloat16

B = 2048
T = 32
TAU = 1024
MSG = 512
HID = 1024
VDIM = 128
N_CORES = 8
B_LOCAL = B // N_CORES

Alu = mybir.AluOpType
Act = mybir.ActivationFunctionType


def build(b_local=B_LOCAL, t_chunk=8, chunk_bufs=4, n_scalar_red=3):
    assert b_local % 128 == 0 and T % t_chunk == 0
    n_btiles = b_local // 128
    n_chunks = T // t_chunk
    chunk_free = t_chunk * TAU

    nc = bacc.Bacc("TRN2", target_bir_lowering=False, debug=False)

    traj = nc.declare_dram_parameter(
        "imagined_trajectory", [b_local, T * TAU], F32, isOutput=False
    )
    msgsT = nc.declare_dram_parameter("msgsT", [MSG, b_local], BF16, isOutput=False)
    Wfused = nc.declare_dram_parameter("Wfused", [MSG, TAU], BF16, isOutput=False)
    qkbias = nc.declare_dram_parameter("qkbias", [TAU], BF16, isOutput=False)
    WvT = nc.declare_dram_parameter("WvT", [TAU, VDIM], BF16, isOutput=False)
    bv = nc.declare_dram_parameter("bv", [VDIM], F32, isOutput=False)
    out = nc.declare_dram_parameter("out", [b_local, VDIM], F32, isOutput=True)

    MQ = MSG // 128  # 4 m-chunks
    CQ = TAU // 128  # 8 c-chunks

    with tile.TileContext(nc) as tc:
        with (
            tc.tile_pool(name="const", bufs=1) as const,
            tc.tile_pool(name="persist", bufs=1) as persist,
            tc.tile_pool(name="psum_big", bufs=2, space="PSUM") as psum_big,
            tc.tile_pool(name="psum_tr", bufs=2, space="PSUM") as psum_tr,
            tc.tile_pool(name="psum_out", bufs=2, space="PSUM") as psum_out,
        ):
            ident_f = const.tile([128, 128], F32)
            make_identity(nc, ident_f)
            ident_b = const.tile([128, 128], BF16)
            make_identity(nc, ident_b)
            onespad_b = const.tile([128, 128], BF16)
            nc.vector.memset(onespad_b, 0.0)
            nc.vector.memset(onespad_b[0:1, :], 1.0)
            bv_sb = const.tile([1, VDIM], F32)
            nc.sync.dma_start(out=bv_sb, in_=bv[None, :])
            bvpad_b = const.tile([128, VDIM], BF16)
            nc.vector.memset(bvpad_b, 0.0)
            nc.vector.tensor_copy(out=bvpad_b[0:1, :], in_=bv_sb)
            ones_row = const.tile([1, b_local], BF16)
            nc.vector.memset(ones_row, 1.0)

            # pre-packed weights (host already fused/transposed/cast)
            Wfused_b = const.tile([128, MQ, TAU], BF16)
            nc.sync.dma_start(
                out=Wfused_b, in_=Wfused[:, :].rearrange("(j p) c -> p j c", p=128)
            )
            msgsT_b = const.tile([128, MQ, b_local], BF16)
            nc.sync.dma_start(
                out=msgsT_b, in_=msgsT[:, :].rearrange("(j p) b -> p j b", p=128)
            )
            WvT_b = const.tile([128, CQ, VDIM], BF16)
            nc.sync.dma_start(
                out=WvT_b, in_=WvT[:, :].rearrange("(j p) d -> p j d", p=128)
            )
            qkb_sb = const.tile([1, TAU], BF16)
            nc.sync.dma_start(out=qkb_sb, in_=qkbias[None, :])

            # qk[b, c] = msgs @ Wfused + qk_bias, scaled by 1/sqrt(H)
            qk_b = [
                persist.tile([128, TAU], BF16, tag=f"qkb{i}", name=f"qkb{i}")
                for i in range(n_btiles)
            ]
            for bi in range(n_btiles):
                bsl = slice(bi * 128, (bi + 1) * 128)
                pq = psum_big.tile([128, TAU], F32, tag="ctx", name="pq")
                for nh in range(2):
                    nsl = slice(nh * 512, (nh + 1) * 512)
                    for mi in range(MQ):
                        nc.tensor.matmul(
                            pq[:, nsl],
                            lhsT=msgsT_b[:, mi, bsl],
                            rhs=Wfused_b[:, mi, nsl],
                            start=(mi == 0),
                            stop=False,
                        )
                    nc.tensor.matmul(
                        pq[:, nsl],
                        lhsT=ones_row[:, bsl],
                        rhs=qkb_sb[:, nsl],
                        start=False,
                        stop=True,
                    )
                nc.scalar.mul(out=qk_b[bi], in_=pq, mul=1.0 / math.sqrt(HID))

            # ---------- main loop: stream tau ----------
            with (
                tc.tile_pool(name="stream", bufs=chunk_bufs) as stream,
                tc.tile_pool(name="bfp", bufs=2) as bfp,
                tc.tile_pool(name="dpool", bufs=2) as dpool,
                tc.tile_pool(name="spool", bufs=4) as spool,
                tc.tile_pool(name="aux", bufs=2) as aux,
            ):
                dumm = aux.tile([128, TAU], BF16, tag="dumm", name="dumm", bufs=1)
                for bi in range(n_btiles):
                    bsl = slice(bi * 128, (bi + 1) * 128)
                    ctx_ps = psum_big.tile([128, TAU], F32, tag="ctx", name="ctx_ps")
                    p_all = aux.tile([128, T], F32, tag="p", name="p_all")

                    for ci in range(n_chunks):
                        chunk_bf = stream.tile(
                            [128, chunk_free], BF16, tag="chunk", name="chunk_bf"
                        )
                        c0 = ci * chunk_free
                        nc.gpsimd.dma_start(
                            out=chunk_bf, in_=traj[bsl, c0 : c0 + chunk_free]
                        )
                        # prod = chunk * qk (broadcast over t) in one DVE pass
                        prod = bfp.tile(
                            [128, chunk_free], BF16, tag="prod", name="prod"
                        )
                        qk_rep = bass.AP(
                            tensor=qk_b[bi].tensor,
                            offset=qk_b[bi].offset,
                            ap=[qk_b[bi].ap[0], [0, t_chunk], [1, TAU]],
                        )
                        nc.vector.tensor_tensor(
                            out=prod, in0=chunk_bf, in1=qk_rep, op=Alu.mult
                        )
                        # scores: first n_scalar_red slices on scalar (activation
                        # accumulator), the rest on vector (tensor_reduce)
                        scores_sc = spool.tile(
                            [128, n_scalar_red], F32, tag="ssc", name="scores_sc"
                        )
                        scores_ve = spool.tile(
                            [128, t_chunk - n_scalar_red], F32, tag="sve",
                            name="scores_ve",
                        )
                        for tt in range(t_chunk):
                            seg = slice(tt * TAU, (tt + 1) * TAU)
                            if tt < n_scalar_red:
                                nc.scalar.activation(
                                    out=dumm,
                                    in_=prod[:, seg],
                                    func=Act.Copy,
                                    accum_out=scores_sc[:, tt : tt + 1],
                                )
                            else:
                                j = tt - n_scalar_red
                                nc.vector.tensor_reduce(
                                    out=scores_ve[:, j : j + 1],
                                    in_=prod[:, seg],
                                    axis=mybir.AxisListType.X,
                                    op=Alu.add,
                                )
                        c0t = ci * t_chunk
                        csl = slice(c0t, c0t + t_chunk)
                        nc.scalar.activation(
                            out=p_all[:, c0t : c0t + n_scalar_red],
                            in_=scores_sc,
                            func=Act.Exp,
                        )
                        nc.scalar.activation(
                            out=p_all[:, c0t + n_scalar_red : c0t + t_chunk],
                            in_=scores_ve,
                            func=Act.Exp,
                        )
                        # diag blocks: diag_all[:, t, :] = ident * p[:, t]
                        diag_all = dpool.tile(
                            [128, t_chunk, 128], BF16, tag="diag", name="diag_all"
                        )
                        ident_rep = bass.AP(
                            tensor=ident_b.tensor,
                            offset=ident_b.offset,
                            ap=[ident_b.ap[0], [0, t_chunk], [1, 128]],
                        )
                        p_sl = p_all[:, csl]
                        p_rep = bass.AP(
                            tensor=p_sl.tensor,
                            offset=p_sl.offset,
                            ap=[p_sl.ap[0], p_sl.ap[1], [0, 128]],
                        )
                        nc.vector.tensor_tensor(
                            out=diag_all, in0=ident_rep, in1=p_rep, op=Alu.mult
                        )
                        for tt in range(t_chunk):
                            first = ci == 0 and tt == 0
                            last = ci == n_chunks - 1 and tt == t_chunk - 1
                            for nh in range(2):
                                nsl = slice(
                                    tt * TAU + nh * 512, tt * TAU + (nh + 1) * 512
                                )
                                nc.tensor.matmul(
                                    ctx_ps[:, nh * 512 : (nh + 1) * 512],
                                    lhsT=diag_all[:, tt, :],
                                    rhs=chunk_bf[:, nsl],
                                    start=first,
                                    stop=last,
                                )

                    # normalize, project: out = (ctx / sum p) @ Wv.T + bv
                    s_sum = aux.tile([128, 1], F32, tag="ssum", name="s_sum")
                    nc.vector.tensor_reduce(
                        out=s_sum, in_=p_all, axis=mybir.AxisListType.X, op=Alu.add
                    )
                    rinv = aux.tile([128, 1], F32, tag="rinv", name="rinv")
                    nc.vector.reciprocal(out=rinv, in_=s_sum)
                    ctxn_f = aux.tile([128, TAU], F32, tag="ctxn", name="ctxn_f")
                    nc.scalar.activation(
                        out=ctxn_f, in_=ctx_ps, func=Act.Copy, scale=rinv
                    )
                    ctxT_b = aux.tile([128, CQ, 128], BF16, tag="ctxT", name="ctxT_b")
                    for j in range(CQ):
                        ptb = psum_tr.tile([128, 128], F32, tag="tr", name="ptb")
                        nc.tensor.transpose(
                            ptb, ctxn_f[:, j * 128 : (j + 1) * 128], ident_f
                        )
                        nc.scalar.copy(out=ctxT_b[:, j, :], in_=ptb)
                    pm = psum_out.tile([128, VDIM], F32, tag="mm", name="pm")
                    for j in range(CQ):
                        nc.tensor.matmul(
                            pm,
                            lhsT=ctxT_b[:, j, :],
                            rhs=WvT_b[:, j, :],
                            start=(j == 0),
                            stop=False,
                        )
                    nc.tensor.matmul(
                        pm, lhsT=onespad_b, rhs=bvpad_b, start=False, stop=True
                    )
                    msg_out = aux.tile([128, VDIM], F32, tag="msg", name="msg_out")
                    nc.scalar.copy(out=msg_out, in_=pm)
                    nc.sync.dma_start(out=out[bsl, :], in_=msg_out)

    nc.compile()
    return nc


_NC_CACHE = {}


def _get_nc():
    key = "default"
    if key not in _NC_CACHE:
        _NC_CACHE[key] = build()
    return _NC_CACHE[key]


def make_in_maps(imagined_trajectory, received_messages, Wq, bq, Wk, Wv, bv):
    Wq = np.asarray(Wq, dtype=np.float32)
    bq = np.asarray(bq, dtype=np.float32)
    Wk = np.asarray(Wk, dtype=np.float32)
    Wv = np.asarray(Wv, dtype=np.float32)
    bv = np.asarray(bv, dtype=np.float32)
    # batch-independent weight fusion, done once on host
    Wfused = np.ascontiguousarray(Wq.T @ Wk).astype(NP_BF16)  # [MSG, TAU]
    qkbias = (bq @ Wk).astype(NP_BF16)  # [TAU]
    WvT = np.ascontiguousarray(Wv.T).astype(NP_BF16)  # [TAU, VDIM]

    bl = B_LOCAL
    in_maps = []
    for i in range(N_CORES):
        sl = slice(i * bl, (i + 1) * bl)
        in_maps.append(
            {
                "imagined_trajectory": np.ascontiguousarray(
                    imagined_trajectory[sl], dtype=np.float32
                ),
                "msgsT": np.ascontiguousarray(
                    np.asarray(received_messages[sl], dtype=np.float32).T
                ).astype(NP_BF16),
                "Wfused": Wfused,
                "qkbias": qkbias,
                "WvT": WvT,
                "bv": bv,
            }
        )
    return in_maps


def kernel(
    imagined_trajectory,
    received_messages,
    Wq,
    bq,
    Wk,
    bk,
    Wv,
    bv,
):
    nc = _get_nc()
    in_maps = make_in_maps(
        imagined_trajectory, received_messages, Wq, bq, Wk, Wv, bv
    )
    res = run_bass_kernel_spmd(nc, in_maps, list(range(N_CORES)))
    return np.concatenate([res.results[i]["out"] for i in range(N_CORES)], axis=0)


# revision 6
# speedup vs baseline: 1.5341x; 1.3660x over previous
"""Trainium2 Bass kernel for nn_AttentionMechanism (batched attention with
per-sample queries), data-parallel across 8 NeuronCores.

Math (per batch row b):
    q = msgs @ Wq.T + bq                         [H]
    k_t = Wk @ tau_t + bk ; scores_t = q.k_t/32
    alpha = softmax(scores) ; out = sum_t alpha_t (Wv @ tau_t + bv)

Rewrite used (exact up to softmax shift invariance):
    qk   = msgs @ (Wq.T @ Wk) + bq @ Wk          [TAU]   (q.bk const in t -> cancels)
    scores_t = qk . tau_t / 32
    p_t  = exp(scores_t)            (scores are O(1), no max-subtraction needed)
    ctx  = sum_t p_t tau_t / sum_t p_t
    out  = ctx @ Wv.T + bv          (uses sum alpha = 1)

Host precomputes the batch-independent weight products (Wfused = Wq.T @ Wk,
qk_bias = bq @ Wk, WvT = Wv.T) and packs them bf16, so the device streams tau
once from HBM (32 MB/core) plus ~1.5 MB of weights.

Device schedule per 128-row b-tile, per t-chunk of 8 trajectory steps:
  DMA   : chunk [128, 8, 1024] f32->bf16 cast on load (~11.1 us, the bound)
  Vector: 1x broadcast mult prod = chunk * qk_rep; 1x 3D tensor_reduce for the
          last 5 scores
  Scalar: 3x activation-accum for the first 3 scores; 2x exp;
          8x diag build (diag_t = ident * p_t via activation scale)
  PE    : 16x matmul ctx[:, bank] += diag(p_t) @ chunk_t  (p_t scaling folded
          into the matmul weights, so no separate p_t*tau elementwise pass)
"""

import math

import numpy as np
import ml_dtypes

import concourse.bass as bass
import concourse.bacc as bacc
import concourse.tile as tile
from concourse import mybir
from concourse.bass_utils import run_bass_kernel_spmd
from concourse.masks import make_identity

F32 = mybir.dt.float32
BF16 = mybir.dt.bfloat16
NP_BF16 = ml_dtypes.bfloat16

B = 2048
T = 32
TAU = 1024
MSG = 512
HID = 1024
VDIM = 128
N_CORES = 8
B_LOCAL = B // N_CORES

Alu = mybir.AluOpType
Act = mybir.ActivationFunctionType


def build(b_local=B_LOCAL, t_chunk=8, chunk_bufs=4, n_scalar_red=3):
    assert b_local % 128 == 0 and T % t_chunk == 0
    n_btiles = b_local // 128
    n_chunks = T // t_chunk

    nc = bacc.Bacc("TRN2", target_bir_lowering=False, debug=False)

    traj = nc.declare_dram_parameter(
        "imagined_trajectory", [b_local, T * TAU], F32, isOutput=False
    )
    msgsT = nc.declare_dram_parameter("msgsT", [MSG, b_local], BF16, isOutput=False)
    Wfused = nc.declare_dram_parameter("Wfused", [MSG, TAU], BF16, isOutput=False)
    qkbias = nc.declare_dram_parameter("qkbias", [TAU], BF16, isOutput=False)
    WvT = nc.declare_dram_parameter("WvT", [TAU, VDIM], BF16, isOutput=False)
    bv = nc.declare_dram_parameter("bv", [VDIM], F32, isOutput=False)
    out = nc.declare_dram_parameter("out", [b_local, VDIM], F32, isOutput=True)

    MQ = MSG // 128  # 4 m-chunks
    CQ = TAU // 128  # 8 c-chunks

    with tile.TileContext(nc) as tc:
        with (
            tc.tile_pool(name="const", bufs=1) as const,
            tc.tile_pool(name="persist", bufs=1) as persist,
            tc.tile_pool(name="psum_big", bufs=2, space="PSUM") as psum_big,
            tc.tile_pool(name="psum_tr", bufs=2, space="PSUM") as psum_tr,
            tc.tile_pool(name="psum_out", bufs=2, space="PSUM") as psum_out,
        ):
            # weights needed for qk first (critical path to first chunk compute)
            Wfused_b = const.tile([128, MQ, TAU], BF16)
            nc.gpsimd.dma_start(
                out=Wfused_b, in_=Wfused[:, :].rearrange("(j p) c -> p j c", p=128)
            )
            msgsT_b = const.tile([128, MQ, b_local], BF16)
            nc.gpsimd.dma_start(
                out=msgsT_b, in_=msgsT[:, :].rearrange("(j p) b -> p j b", p=128)
            )
            qkb_sb = const.tile([1, TAU], BF16)
            nc.gpsimd.dma_start(out=qkb_sb, in_=qkbias[None, :])
            WvT_b = const.tile([128, CQ, VDIM], BF16)
            nc.gpsimd.dma_start(
                out=WvT_b, in_=WvT[:, :].rearrange("(j p) d -> p j d", p=128)
            )
            bv_sb = const.tile([1, VDIM], F32)
            nc.sync.dma_start(out=bv_sb, in_=bv[None, :])

            ident_f = const.tile([128, 128], F32)
            make_identity(nc, ident_f)
            ident_b = const.tile([128, 128], BF16)
            make_identity(nc, ident_b)
            onespad_b = const.tile([128, 128], BF16)
            nc.vector.memset(onespad_b, 0.0)
            nc.vector.memset(onespad_b[0:1, :], 1.0)
            bvpad_b = const.tile([128, VDIM], BF16)
            nc.vector.memset(bvpad_b, 0.0)
            nc.vector.tensor_copy(out=bvpad_b[0:1, :], in_=bv_sb)
            ones_row = const.tile([1, b_local], BF16)
            nc.vector.memset(ones_row, 1.0)

            # qk[b, c] = msgs @ Wfused + qk_bias, scaled by 1/sqrt(H)
            qk_b = [
                persist.tile([128, TAU], BF16, tag=f"qkb{i}", name=f"qkb{i}")
                for i in range(n_btiles)
            ]
            for bi in range(n_btiles):
                bsl = slice(bi * 128, (bi + 1) * 128)
                pq = psum_big.tile([128, TAU], F32, tag="ctx", name="pq")
                for nh in range(2):
                    nsl = slice(nh * 512, (nh + 1) * 512)
                    for mi in range(MQ):
                        nc.tensor.matmul(
                            pq[:, nsl],
                            lhsT=msgsT_b[:, mi, bsl],
                            rhs=Wfused_b[:, mi, nsl],
                            start=(mi == 0),
                            stop=False,
                        )
                    nc.tensor.matmul(
                        pq[:, nsl],
                        lhsT=ones_row[:, bsl],
                        rhs=qkb_sb[:, nsl],
                        start=False,
                        stop=True,
                    )
                nc.scalar.mul(out=qk_b[bi], in_=pq, mul=1.0 / math.sqrt(HID))

            # ---------- main loop: stream tau ----------
            with (
                tc.tile_pool(name="stream", bufs=chunk_bufs) as stream,
                tc.tile_pool(name="bfp", bufs=2) as bfp,
                tc.tile_pool(name="dpool", bufs=2) as dpool,
                tc.tile_pool(name="spool", bufs=4) as spool,
                tc.tile_pool(name="aux", bufs=2) as aux,
            ):
                dumm = aux.tile([128, TAU], BF16, tag="dumm", name="dumm", bufs=1)
                n_vec_red = t_chunk - n_scalar_red
                for bi in range(n_btiles):
                    bsl = slice(bi * 128, (bi + 1) * 128)
                    ctx_ps = psum_big.tile([128, TAU], F32, tag="ctx", name="ctx_ps")
                    p_all = aux.tile([128, T], F32, tag="p", name="p_all")

                    for ci in range(n_chunks):
                        chunk_bf = stream.tile(
                            [128, t_chunk, TAU], BF16, tag="chunk", name="chunk_bf"
                        )
                        c0 = ci * t_chunk * TAU
                        nc.gpsimd.dma_start(
                            out=chunk_bf,
                            in_=traj[bsl, c0 : c0 + t_chunk * TAU].rearrange(
                                "p (t c) -> p t c", t=t_chunk
                            ),
                        )
                        # prod = chunk * qk (broadcast over t) in one DVE pass
                        prod = bfp.tile(
                            [128, t_chunk, TAU], BF16, tag="prod", name="prod"
                        )
                        qk_rep = bass.AP(
                            tensor=qk_b[bi].tensor,
                            offset=qk_b[bi].offset,
                            ap=[qk_b[bi].ap[0], [0, t_chunk], [1, TAU]],
                        )
                        nc.vector.tensor_tensor(
                            out=prod, in0=chunk_bf, in1=qk_rep, op=Alu.mult
                        )
                        # scores: first n_scalar_red slices on scalar (activation
                        # accumulator), the rest in one 3D vector tensor_reduce
                        scores_sc = spool.tile(
                            [128, n_scalar_red], F32, tag="ssc", name="scores_sc"
                        )
                        scores_ve = spool.tile(
                            [128, n_vec_red], F32, tag="sve", name="scores_ve"
                        )
                        for tt in range(n_scalar_red):
                            nc.scalar.activation(
                                out=dumm,
                                in_=prod[:, tt, :],
                                func=Act.Copy,
                                accum_out=scores_sc[:, tt : tt + 1],
                            )
                        nc.vector.tensor_reduce(
                            out=scores_ve,
                            in_=prod[:, n_scalar_red:, :],
                            axis=mybir.AxisListType.X,
                            op=Alu.add,
                        )
                        c0t = ci * t_chunk
                        nc.scalar.activation(
                            out=p_all[:, c0t : c0t + n_scalar_red],
                            in_=scores_sc,
                            func=Act.Exp,
                        )
                        nc.scalar.activation(
                            out=p_all[:, c0t + n_scalar_red : c0t + t_chunk],
                            in_=scores_ve,
                            func=Act.Exp,
                        )
                        # diag blocks on scalar: diag_all[:, t, :] = ident * p_t
                        diag_all = dpool.tile(
                            [128, t_chunk, 128], BF16, tag="diag", name="diag_all"
                        )
                        for tt in range(t_chunk):
                            col = c0t + tt
                            nc.scalar.activation(
                                out=diag_all[:, tt, :],
                                in_=ident_b,
                                func=Act.Copy,
                                scale=p_all[:, col : col + 1],
                            )
                        for tt in range(t_chunk):
                            first = ci == 0 and tt == 0
                            last = ci == n_chunks - 1 and tt == t_chunk - 1
                            for nh in range(2):
                                nc.tensor.matmul(
                                    ctx_ps[:, nh * 512 : (nh + 1) * 512],
                                    lhsT=diag_all[:, tt, :],
                                    rhs=chunk_bf[:, tt, nh * 512 : (nh + 1) * 512],
                                    start=first,
                                    stop=last,
                                )

                    # normalize, project: out = (ctx / sum p) @ Wv.T + bv
                    s_sum = aux.tile([128, 1], F32, tag="ssum", name="s_sum")
                    nc.vector.tensor_reduce(
                        out=s_sum, in_=p_all, axis=mybir.AxisListType.X, op=Alu.add
                    )
                    rinv = aux.tile([128, 1], F32, tag="rinv", name="rinv")
                    nc.vector.reciprocal(out=rinv, in_=s_sum)
                    ctxn_f = aux.tile([128, TAU], F32, tag="ctxn", name="ctxn_f")
                    nc.scalar.activation(
                        out=ctxn_f, in_=ctx_ps, func=Act.Copy, scale=rinv
                    )
                    ctxT_b = aux.tile([128, CQ, 128], BF16, tag="ctxT", name="ctxT_b")
                    for j in range(CQ):
                        ptb = psum_tr.tile([128, 128], F32, tag="tr", name="ptb")
                        nc.tensor.transpose(
                            ptb, ctxn_f[:, j * 128 : (j + 1) * 128], ident_f
                        )
                        nc.scalar.copy(out=ctxT_b[:, j, :], in_=ptb)
                    pm = psum_out.tile([128, VDIM], F32, tag="mm", name="pm")
                    for j in range(CQ):
                        nc.tensor.matmul(
                            pm,
                            lhsT=ctxT_b[:, j, :],
                            rhs=WvT_b[:, j, :],
                            start=(j == 0),
                            stop=False,
                        )
                    nc.tensor.matmul(
                        pm, lhsT=onespad_b, rhs=bvpad_b, start=False, stop=True
                    )
                    msg_out = aux.tile([128, VDIM], F32, tag="msg", name="msg_out")
                    nc.scalar.copy(out=msg_out, in_=pm)
                    nc.sync.dma_start(out=out[bsl, :], in_=msg_out)

    nc.compile()
    return nc


_NC_CACHE = {}


def _get_nc():
    key = "default"
    if key not in _NC_CACHE:
        _NC_CACHE[key] = build()
    return _NC_CACHE[key]


def make_in_maps(imagined_trajectory, received_messages, Wq, bq, Wk, Wv, bv):
    Wq = np.asarray(Wq, dtype=np.float32)
    bq = np.asarray(bq, dtype=np.float32)
    Wk = np.asarray(Wk, dtype=np.float32)
    Wv = np.asarray(Wv, dtype=np.float32)
    bv = np.asarray(bv, dtype=np.float32)
    # batch-independent weight fusion, done once on host
    Wfused = np.ascontiguousarray(Wq.T @ Wk).astype(NP_BF16)  # [MSG, TAU]
    qkbias = (bq @ Wk).astype(NP_BF16)  # [TAU]
    WvT = np.ascontiguousarray(Wv.T).astype(NP_BF16)  # [TAU, VDIM]

    bl = B_LOCAL
    in_maps = []
    for i in range(N_CORES):
        sl = slice(i * bl, (i + 1) * bl)
        in_maps.append(
            {
                "imagined_trajectory": np.ascontiguousarray(
                    imagined_trajectory[sl], dtype=np.float32
                ),
                "msgsT": np.ascontiguousarray(
                    np.asarray(received_messages[sl], dtype=np.float32).T
                ).astype(NP_BF16),
                "Wfused": Wfused,
                "qkbias": qkbias,
                "WvT": WvT,
                "bv": bv,
            }
        )
    return in_maps


def kernel(
    imagined_trajectory,
    received_messages,
    Wq,
    bq,
    Wk,
    bk,
    Wv,
    bv,
):
    nc = _get_nc()
    in_maps = make_in_maps(
        imagined_trajectory, received_messages, Wq, bq, Wk, Wv, bv
    )
    res = run_bass_kernel_spmd(nc, in_maps, list(range(N_CORES)))
    return np.concatenate([res.results[i]["out"] for i in range(N_CORES)], axis=0)
